# revision 1
# baseline (speedup 1.0000x reference)
"""Trainium2 Bass kernel for the gnn_message_passing problem.

Math refactor: the reference computes
    kernel[z,i,j] = einsum('zk,kij->zij', Rk*Yk, cg) * nc0[i,j]
with Rk = R @ rf_mix.T (rank 6) and Yk = Y.T @ ylm_mix.T (rank 9).
Since Rk*Yk has rank<=54 over k, fold the K=1024 contraction into a
precomputed M[p*9+l, ij] = sum_k rf[k,p]*ylm_s[k,l]*cg[k,ij] * nc0[ij]
(computed on device from the cg/rf/ylm/norm inputs), and per point only
contract B[z, pl] = R[z,p]*Y'[z,l] against M - a k=54 matmul. This cuts
compute ~20x and makes the kernel memory-bound (410 MB output).

Distribution: data-parallel over z across 8 NeuronCores; constants
replicated. Full inputs in, full output out.

Precision: the fast fp32r matmul path rounds inputs to an 11-bit
mantissa, so the main contraction uses a hi/lo split:
    out = [Bh;Bl] @ [Mh;Mh]  (k=108, accumulating)  +  Bh @ Ml  (k=54)
which drops only the Bl@Ml term (~2^-24 relative) - fp32-class accuracy
at 1 cycle/row. Everything feeding B (monomials, radial MLP) runs in
exact fp32 (PE fp32 mode, DVE reciprocal, ACT sqrt + one Newton step).
"""

import numpy as np

import concourse.bass as bass
import concourse.tile as tile
from concourse import bacc, mybir
from concourse.bass_utils import run_bass_kernel_spmd

F32 = mybir.dt.float32
F32R = mybir.dt.float32r
ALU = mybir.AluOpType
ACTF = mybir.ActivationFunctionType

# Problem shape (hardcoded per contract)
Z, KDIM, DO, DI, NPATH, H = 100000, 1024, 32, 32, 6, 128
IJ = DO * DI                      # 1024
NCORES = 8
ZC = Z // NCORES                  # 12500 points per core
T = 100                           # point tiles of 128 -> ZC padded to 12800
ZC_PAD = 128 * T
TB = 4                            # tiles per group
NG = T // TB                      # 25 groups
NCH = 10                          # channels: radii, ones, 8 scaled monomials
NKT = KDIM // 128                 # 8 k-tiles for the M build

# Real spherical harmonic constants (l=0,1,2), folded into ylm host-side
C0 = 0.28209479177387814
C1 = 0.4886025119029199
C2A = 1.0925484305920792
C2B = 0.31539156525252005
C2C = 0.5462742152960396
YLM_SCALE = np.array([C0, C1, C1, C1, C2A, C2A, C2B, C2A, C2C], dtype=np.float64)

_CACHE = {}


def _build_program():
    nc = bacc.Bacc("TRN2", target_bir_lowering=False, debug=False,
                   num_devices=NCORES)

    # ---- per-core DRAM I/O ----
    rpl = nc.dram_tensor("rpl", [128, 3 * T], F32, kind="ExternalInput").ap()
    w1e4 = nc.dram_tensor("w1e4", [NCH * TB, TB * 128], F32, kind="ExternalInput").ap()
    ey4 = nc.dram_tensor("ey4", [NCH * TB, TB * 54], F32, kind="ExternalInput").ap()
    w2e = nc.dram_tensor("w2e", [H, 54], F32, kind="ExternalInput").ap()
    b1c = nc.dram_tensor("b1c", [H, 1], F32, kind="ExternalInput").ap()
    b2r = nc.dram_tensor("b2r", [54, 1], F32, kind="ExternalInput").ap()
    cgd = nc.dram_tensor("cgd", [KDIM, IJ], F32, kind="ExternalInput").ap()
    rft = nc.dram_tensor("rft", [128, NKT * NPATH], F32, kind="ExternalInput").ap()
    ylt = nc.dram_tensor("ylt", [128, NKT * 9], F32, kind="ExternalInput").ap()
    ncv = nc.dram_tensor("ncv", [1, IJ], F32, kind="ExternalInput").ap()
    identd = nc.dram_tensor("identd", [128, 128], F32, kind="ExternalInput").ap()
    out = nc.dram_tensor("out", [ZC, IJ], F32, kind="ExternalOutput").ap()

    with tile.TileContext(nc) as tc:
        with tc.tile_pool(name="const", bufs=1) as cpool, \
             tc.tile_pool(name="mbuf", bufs=1) as mpool:
            # ---- resident constants ----
            w1e_sb = cpool.tile([NCH * TB, TB * 128], F32)
            nc.sync.dma_start(w1e_sb[:], w1e4[:])
            ey4_sb = cpool.tile([NCH * TB, TB * 54], F32)
            nc.sync.dma_start(ey4_sb[:], ey4[:])
            w2e_sb = cpool.tile([H, 54], F32)
            nc.sync.dma_start(w2e_sb[:], w2e[:])
            b1_sb = cpool.tile([H, 1], F32)
            nc.sync.dma_start(b1_sb[:], b1c[:])
            b2_sb = cpool.tile([54, 1], F32)
            nc.sync.dma_start(b2_sb[:], b2r[:])
            id_sb = cpool.tile([128, 128], F32)
            nc.sync.dma_start(id_sb[:], identd[:])
            ncv_sb = cpool.tile([1, IJ], F32)
            nc.sync.dma_start(ncv_sb[:], ncv[:])
            ones54 = cpool.tile([1, 54], F32)
            nc.vector.memset(ones54[:], 1.0)

            # M tensors: rows 0-53 = Mh, 54-63 = zeros (partition-alignment
            # filler; engine writes must start at 0/32/64/96), 64-117 = Mh.
            mstk = cpool.tile([118, IJ], F32R)
            ml_sb = cpool.tile([54, IJ], F32R)
            # B hi/lo stack: 3 manually rotated slots along the free dim
            bstk_all = cpool.tile([118, 6 * 128], F32R)
            nc.vector.memset(mstk[32:64, :].bitcast(F32), 0.0)
            nc.vector.memset(bstk_all[32:64, :].bitcast(F32), 0.0)

            # =========================================================
            # Phase 1: build M[pl, ij] from cg, rf, ylm, norm_coef
            # =========================================================
            with tc.tile_pool(name="mpsum", bufs=1, space="PSUM") as mps_pool:
                cg_sb = mpool.tile([128, NKT * IJ], F32)
                cg_r = cgd.rearrange("(kt p) ij -> p kt ij", p=128)
                nc.sync.dma_start(
                    cg_sb[:].rearrange("p (kt ij) -> p kt ij", kt=NKT), cg_r)
                rf_sb = mpool.tile([128, NKT * NPATH], F32)
                nc.sync.dma_start(rf_sb[:], rft[:])
                yl_sb = mpool.tile([128, NKT * 9], F32)
                nc.sync.dma_start(yl_sb[:], ylt[:])

                # W[k, pl] = rf[k,p] * ylm_s[k,l]
                w_sb = mpool.tile([128, NKT * 54], F32)
                for kt in range(NKT):
                    for p in range(NPATH):
                        nc.vector.tensor_scalar(
                            w_sb[:, kt * 54 + p * 9: kt * 54 + p * 9 + 9],
                            yl_sb[:, kt * 9: kt * 9 + 9],
                            rf_sb[:, kt * NPATH + p: kt * NPATH + p + 1],
                            None, ALU.mult)

                m_ps = mps_pool.tile([54, IJ], F32)
                for half in range(2):
                    for kt in range(NKT):
                        nc.tensor.matmul(
                            m_ps[:, half * 512:(half + 1) * 512],
                            w_sb[:, kt * 54:(kt + 1) * 54],
                            cg_sb[:, kt * IJ + half * 512: kt * IJ + half * 512 + 512],
                            start=(kt == 0), stop=(kt == NKT - 1))

                # broadcast norm_coef[...,0] across the 54 partitions
                ncr_ps = mps_pool.tile([54, IJ], F32)
                for half in range(2):
                    nc.tensor.matmul(
                        ncr_ps[:, half * 512:(half + 1) * 512],
                        ones54[:],
                        ncv_sb[:, half * 512:(half + 1) * 512],
                        start=True, stop=True)
                ncr_sb = mpool.tile([54, IJ], F32)
                nc.scalar.copy(ncr_sb[:], ncr_ps[:])

                mf_sb = mpool.tile([54, IJ], F32)
                nc.vector.tensor_tensor(mf_sb[:], m_ps[:], ncr_sb[:], ALU.mult)
                # hi/lo split (fp32r rounding happens on write)
                nc.vector.tensor_copy(mstk[0:54, :], mf_sb[:])
                nc.scalar.copy(mstk[64:118, :], mf_sb[:])
                nc.vector.tensor_tensor(ml_sb[:], mf_sb[:],
                                        mstk[0:54, :].bitcast(F32), ALU.subtract)

            # =========================================================
            # Phase 2: per-point planes [128, T]: radii, ones, monomials
            # =========================================================
            rpl_sb = cpool.tile([128, 3 * T], F32)
            nc.sync.dma_start(rpl_sb[:], rpl[:])
            x = rpl_sb[:, 0:T]
            y = rpl_sb[:, T:2 * T]
            z = rpl_sb[:, 2 * T:3 * T]

            chan = cpool.tile([128, NCH * T], F32)
            aux = cpool.tile([128, 10 * T], F32)

            def ax(i):
                return aux[:, i * T:(i + 1) * T]

            xx, yy, zz, s1, r2, mask, inv2, va, vb, t8 = (ax(i) for i in range(10))
            nc.vector.tensor_tensor(xx, x, x, ALU.mult)
            nc.vector.tensor_tensor(yy, y, y, ALU.mult)
            nc.vector.tensor_tensor(zz, z, z, ALU.mult)
            nc.vector.tensor_tensor(s1, xx, yy, ALU.add)
            nc.vector.tensor_tensor(r2, s1, zz, ALU.add)
            # guard r2 == 0 exactly like the reference's safe_r2
            nc.vector.tensor_scalar(mask, r2, 0.0, None, ALU.is_equal)
            nc.vector.tensor_tensor(mask, r2, mask, ALU.add)     # safe_r2
            nc.vector.reciprocal(inv2, mask)                     # 1/safe_r2 (accurate)
            nc.scalar.sqrt(va, inv2)                             # rsqrt seed ~7e-6
            # one Newton step: v = v*(1.5 - 0.5*safe_r2*v^2)
            nc.vector.tensor_tensor(vb, va, va, ALU.mult)
            nc.vector.tensor_tensor(vb, vb, mask, ALU.mult)
            nc.vector.tensor_scalar(vb, vb, -0.5, 1.5, ALU.mult, ALU.add)
            nc.vector.tensor_tensor(va, va, vb, ALU.mult)        # inv_r

            # chan is stored t-major interleaved (col = t*NCH + c) so each
            # group's transpose input is one contiguous 40-col slice
            chan_v = chan[:].rearrange("p (t c) -> p c t", c=NCH)
            ch = [chan_v[:, i, :] for i in range(NCH)]
            nc.vector.tensor_tensor(ch[0], r2, va, ALU.mult)     # radii
            nc.vector.tensor_scalar(ch[1], r2, 0.0, 1.0, ALU.mult, ALU.add)  # ones
            nc.vector.tensor_tensor(ch[2], y, va, ALU.mult)      # l=1
            nc.vector.tensor_tensor(ch[3], z, va, ALU.mult)      # l=2
            nc.vector.tensor_tensor(ch[4], x, va, ALU.mult)      # l=3
            nc.vector.tensor_tensor(vb, x, y, ALU.mult)
            nc.vector.tensor_tensor(ch[5], vb, inv2, ALU.mult)   # l=4: xy/r2
            nc.vector.tensor_tensor(vb, y, z, ALU.mult)
            nc.vector.tensor_tensor(ch[6], vb, inv2, ALU.mult)   # l=5: yz/r2
            nc.vector.scalar_tensor_tensor(vb, zz, 3.0, r2, ALU.mult, ALU.subtract)
            nc.vector.tensor_tensor(ch[7], vb, inv2, ALU.mult)   # l=6: (3zz-r2)/r2
            nc.vector.tensor_tensor(vb, x, z, ALU.mult)
            nc.vector.tensor_tensor(ch[8], vb, inv2, ALU.mult)   # l=7: xz/r2
            nc.vector.tensor_tensor(t8, xx, yy, ALU.subtract)
            nc.vector.tensor_tensor(ch[9], t8, inv2, ALU.mult)   # l=8: (xx-yy)/r2

            # =========================================================
            # Phase 3: main loop over 25 groups of 4 point-tiles
            # =========================================================
            with tc.tile_pool(name="tps", bufs=2, space="PSUM") as tps_pool, \
                 tc.tile_pool(name="hps", bufs=1, space="PSUM") as hps_pool, \
                 tc.tile_pool(name="rps", bufs=1, space="PSUM") as rps_pool, \
                 tc.tile_pool(name="yps", bufs=1, space="PSUM") as yps_pool, \
                 tc.tile_pool(name="kps", bufs=3, space="PSUM") as kps_pool, \
                 tc.tile_pool(name="work", bufs=2) as wpool, \
                 tc.tile_pool(name="bwork", bufs=4) as bpool, \
                 tc.tile_pool(name="kout", bufs=3) as kpool:
                for g in range(NG):
                    t0 = TB * g
                    # transpose 4 tiles x 10 channels -> [40, 128]
                    t_ps = tps_pool.tile([NCH * TB, 128], F32)
                    nc.tensor.transpose(
                        t_ps[:], chan[:, NCH * t0:NCH * t0 + NCH * TB], id_sb[:])
                    t_sb = wpool.tile([NCH * TB, 128], F32, tag="t_sb")
                    nc.scalar.copy(t_sb[:], t_ps[:])

                    # radial MLP hidden layer for the whole group
                    h_ps = hps_pool.tile([128, TB * 128], F32)
                    for dt in range(TB):
                        nc.tensor.matmul(
                            h_ps[:, dt * 128:(dt + 1) * 128],
                            w1e_sb[:, dt * 128:(dt + 1) * 128],
                            t_sb[:], start=True, stop=True)
                    h_sb = wpool.tile([128, TB * 128], F32, tag="h_sb")
                    nc.scalar.activation(h_sb[:], h_ps[:], ACTF.Relu, bias=b1_sb[:])

                    r_ps = rps_pool.tile([54, TB * 128], F32)
                    nc.tensor.matmul(r_ps[:], w2e_sb[:], h_sb[:],
                                     start=True, stop=True)
                    y_ps = yps_pool.tile([54, TB * 128], F32)
                    for dt in range(TB):
                        nc.tensor.matmul(
                            y_ps[:, dt * 128:(dt + 1) * 128],
                            ey4_sb[:, dt * 54:(dt + 1) * 54],
                            t_sb[:], start=True, stop=True)

                    # B = (R + b2) * Y', split hi/lo for the fp32r contraction
                    b1g = wpool.tile([54, TB * 128], F32, tag="b1g")
                    nc.vector.tensor_scalar(b1g[:], r_ps[:], b2_sb[:],
                                            None, ALU.add)

                    k_sb = kpool.tile([128, TB * IJ], F32, tag="k_sb")
                    for dt in range(TB):
                        bf = bpool.tile([54, 128], F32, tag="bf")
                        nc.vector.tensor_tensor(
                            bf[:], b1g[:, dt * 128:(dt + 1) * 128],
                            y_ps[:, dt * 128:(dt + 1) * 128], ALU.mult)
                        slot = (g * TB + dt) % 6
                        bstk = bstk_all[:, slot * 128:(slot + 1) * 128]
                        nc.vector.tensor_copy(bstk[0:54, :], bf[:])
                        nc.vector.tensor_tensor(
                            bstk[64:118, :], bf[:],
                            bstk[0:54, :].bitcast(F32), ALU.subtract)

                        for half in range(2):
                            k_ps = kps_pool.tile([128, 512], F32, tag="k_ps")
                            nc.tensor.matmul(
                                k_ps[:], bstk[:],
                                mstk[:, half * 512:(half + 1) * 512],
                                start=True, stop=False)
                            nc.tensor.matmul(
                                k_ps[:], bstk[0:54, :],
                                ml_sb[:, half * 512:(half + 1) * 512],
                                start=False, stop=True)
                            dest = k_sb[:, dt * IJ + half * 512:
                                        dt * IJ + (half + 1) * 512]
                            if (dt * 2 + half) % 4 == 3:
                                nc.vector.tensor_copy(dest, k_ps[:])
                            else:
                                nc.scalar.copy(dest, k_ps[:])

                    # store: group covers z rows [512g, 512g+512)
                    z0 = 512 * g
                    if z0 + 512 <= ZC:
                        for hfg in range(2):
                            og = out[z0 + hfg * 256:z0 + hfg * 256 + 256, :].rearrange(
                                "(dt pg) ij -> pg dt ij", dt=2)
                            nc.sync.dma_start(
                                og, k_sb[:, hfg * 2 * IJ:(hfg + 1) * 2 * IJ].rearrange(
                                    "pg (dt ij) -> pg dt ij", dt=2))
                    else:
                        # last group: tiles beyond ZC are padding
                        for dt in range(TB):
                            zt = z0 + dt * 128
                            if zt >= ZC:
                                break
                            rows = min(128, ZC - zt)
                            nc.sync.dma_start(
                                out[zt:zt + rows, :],
                                k_sb[0:rows, dt * IJ:(dt + 1) * IJ])
    nc.compile()
    return nc


def _get_program():
    if "nc" not in _CACHE:
        _CACHE["nc"] = _build_program()
    return _CACHE["nc"]


def _host_prep(r, W1, b1, W2, b2, cg, ylm_mix, rf_mix, norm_coef):
    r = np.asarray(r, dtype=np.float32)
    W1 = np.asarray(W1, dtype=np.float32)
    b1 = np.asarray(b1, dtype=np.float32)
    W2 = np.asarray(W2, dtype=np.float32)
    b2 = np.asarray(b2, dtype=np.float32)
    cg = np.asarray(cg, dtype=np.float32)
    ylm_mix = np.asarray(ylm_mix, dtype=np.float32)
    rf_mix = np.asarray(rf_mix, dtype=np.float32)
    norm_coef = np.asarray(norm_coef, dtype=np.float32)

    w1e4 = np.zeros((NCH * TB, TB * 128), dtype=np.float32)
    ey4 = np.zeros((NCH * TB, TB * 54), dtype=np.float32)
    for dt in range(TB):
        w1e4[NCH * dt, dt * 128:(dt + 1) * 128] = W1[0]
        for l in range(9):
            for p in range(NPATH):
                ey4[NCH * dt + 1 + l, dt * 54 + p * 9 + l] = 1.0

    ylm_s = (ylm_mix.astype(np.float64) * YLM_SCALE[None, :]).astype(np.float32)
    shared = {
        "w1e4": w1e4,
        "ey4": ey4,
        "w2e": np.ascontiguousarray(np.repeat(W2, 9, axis=1)),
        "b1c": np.ascontiguousarray(b1.reshape(H, 1)),
        "b2r": np.ascontiguousarray(np.repeat(b2, 9).reshape(54, 1)),
        "cgd": np.ascontiguousarray(cg.reshape(KDIM, IJ)),
        "rft": np.ascontiguousarray(
            rf_mix.reshape(NKT, 128, NPATH).transpose(1, 0, 2).reshape(128, NKT * NPATH)),
        "ylt": np.ascontiguousarray(
            ylm_s.reshape(NKT, 128, 9).transpose(1, 0, 2).reshape(128, NKT * 9)),
        "ncv": np.ascontiguousarray(norm_coef[:, :, 0].reshape(1, IJ)),
        "identd": np.eye(128, dtype=np.float32),
    }

    in_maps = []
    for c in range(NCORES):
        rs = r[c * ZC:(c + 1) * ZC]
        rp = np.empty((ZC_PAD, 3), dtype=np.float32)
        rp[:ZC] = rs
        rp[ZC:] = np.array([1.0, 0.0, 0.0], dtype=np.float32)
        rpl = rp.reshape(T, 128, 3).transpose(1, 2, 0).reshape(128, 3 * T)
        m = dict(shared)
        m["rpl"] = np.ascontiguousarray(rpl)
        in_maps.append(m)
    return in_maps


def _run_device(in_maps, trace=False, **kw):
    nc = _get_program()
    return run_bass_kernel_spmd(nc, in_maps, core_ids=list(range(NCORES)),
                                trace=trace, **kw)


def kernel(r, W1, b1, W2, b2, cg, ylm_mix, rf_mix, norm_coef):
    r = np.asarray(r, dtype=np.float32)
    norm_coef_f = np.asarray(norm_coef, dtype=np.float32)
    in_maps = _host_prep(r, W1, b1, W2, b2, cg, ylm_mix, rf_mix, norm_coef_f)
    res = _run_device(in_maps)
    out = np.concatenate([res.results[c]["out"] for c in range(NCORES)], axis=0)

    # points with exactly zero radius use norm_coef[..., 1] instead of [..., 0]
    x, y, z = r[:, 0], r[:, 1], r[:, 2]
    r2 = (x * x + y * y) + z * z
    zero = r2 == np.float32(0.0)
    if np.any(zero):
        scale = (norm_coef_f[:, :, 1].astype(np.float64)
                 / norm_coef_f[:, :, 0].astype(np.float64)).reshape(1, IJ)
        out[zero] = (out[zero].astype(np.float64) * scale).astype(np.float32)

    return out.reshape(Z, DO, DI)



# revision 14
# speedup vs baseline: 1.3108x; 1.3108x over previous
"""Trainium2 Bass kernel for the gnn_message_passing problem.

Math refactor: the reference computes
    kernel[z,i,j] = einsum('zk,kij->zij', Rk*Yk, cg) * nc0[i,j]
with Rk = R @ rf_mix.T (rank 6) and Yk = Y.T @ ylm_mix.T (rank 9).
Since Rk*Yk has rank<=54 over k, fold the K=1024 contraction into a
precomputed M[p*9+l, ij] = sum_k rf[k,p]*ylm_s[k,l]*cg[k,ij] * nc0[ij]
(constant folding done host-side in float64), and per point only
contract B[z, pl] = R[z,p]*Y'[z,l] against M - a k=54 matmul. This cuts
compute ~20x and makes the kernel memory-bound (410 MB output).

Distribution: data-parallel over z across 8 NeuronCores; constants
replicated. Full inputs in, full output out.

Performance notes: all matmuls run in fp32r with a moving free dim of
512 (1 cycle/row on the PE). Per 512-point group: 4 PE transposes give
the channel block in [10, 512] layout, one matmul each for the radial
hidden layer, R, and Y', a fused DVE (R+b2)*Y', then per 128-point tile
a k=54 fp32r contraction against M and a PSUM->SBUF->HBM store. The
output DMA (51.2 MB/core) is the roofline; everything else overlaps.
fp32r rounds inputs to an 11-bit mantissa (~2e-4 relative) - well
inside tolerance.
"""

import numpy as np

import concourse.bass as bass
import concourse.tile as tile
from concourse import bacc, mybir
from concourse.bass_utils import run_bass_kernel_spmd

F32 = mybir.dt.float32
F32R = mybir.dt.float32r
ALU = mybir.AluOpType
ACTF = mybir.ActivationFunctionType

# Problem shape (hardcoded per contract)
Z, KDIM, DO, DI, NPATH, H = 100000, 1024, 32, 32, 6, 128
IJ = DO * DI                      # 1024
NCORES = 8
ZC = Z // NCORES                  # 12500 points per core
T = 100                           # point tiles of 128 -> ZC padded to 12800
ZC_PAD = 128 * T
TB = 4                            # tiles per group
NG = T // TB                      # 25 groups
NCH = 10                          # channels: radii, ones, 8 scaled monomials
PL = NPATH * 9                    # 54 = rank of the per-point factor

# Real spherical harmonic constants (l=0,1,2), folded into M host-side
C0 = 0.28209479177387814
C1 = 0.4886025119029199
C2A = 1.0925484305920792
C2B = 0.31539156525252005
C2C = 0.5462742152960396
YLM_SCALE = np.array([C0, C1, C1, C1, C2A, C2A, C2B, C2A, C2C], dtype=np.float64)

_CACHE = {}


def _build_program():
    nc = bacc.Bacc("TRN2", target_bir_lowering=False, debug=False,
                   num_devices=NCORES)

    # ---- per-core DRAM I/O ----
    rpl = nc.dram_tensor("rpl", [128, 3 * T], F32, kind="ExternalInput").ap()
    w1r = nc.dram_tensor("w1r", [NCH, 128], F32R, kind="ExternalInput").ap()
    eyd = nc.dram_tensor("eyd", [NCH, PL], F32R, kind="ExternalInput").ap()
    w2e = nc.dram_tensor("w2e", [H, PL], F32R, kind="ExternalInput").ap()
    b1c = nc.dram_tensor("b1c", [H, 1], F32, kind="ExternalInput").ap()
    b2r = nc.dram_tensor("b2r", [PL, 1], F32, kind="ExternalInput").ap()
    md = nc.dram_tensor("md", [PL, IJ], F32R, kind="ExternalInput").ap()
    identd = nc.dram_tensor("identd", [128, 128], F32, kind="ExternalInput").ap()
    out = nc.dram_tensor("out", [ZC, IJ], F32, kind="ExternalOutput").ap()

    with tile.TileContext(nc) as tc:
        with tc.tile_pool(name="const", bufs=1) as cpool:
            # r planes first (phase 1 gates the whole pipeline); all loads on
            # the otherwise-idle SP queue, in consumption order
            rpl_sb = cpool.tile([128, 3 * T], F32)
            nc.sync.dma_start(rpl_sb[:], rpl[:])
            # ---- resident constants ----
            id_sb = cpool.tile([128, 128], F32)
            nc.sync.dma_start(id_sb[:], identd[:])
            m_sb = cpool.tile([PL, IJ], F32R)
            nc.sync.dma_start(m_sb[:], md[:])
            w1r_sb = cpool.tile([NCH, 128], F32R)
            nc.sync.dma_start(w1r_sb[:], w1r[:])
            ey_sb = cpool.tile([NCH, PL], F32R)
            nc.sync.dma_start(ey_sb[:], eyd[:])
            w2e_sb = cpool.tile([H, PL], F32R)
            nc.sync.dma_start(w2e_sb[:], w2e[:])
            b1_sb = cpool.tile([H, 1], F32)
            nc.sync.dma_start(b1_sb[:], b1c[:])
            b2_sb = cpool.tile([PL, 1], F32)
            nc.sync.dma_start(b2_sb[:], b2r[:])

            # =========================================================
            # Phase 1: per-point planes [128, T]: radii, ones, monomials
            # (emitted in two column chunks so group 0 unblocks early)
            # =========================================================
            chan = cpool.tile([128, NCH * T], F32)
            aux = cpool.tile([128, 10 * T], F32)

            def compute_chan(lo, hi, eng):
                s = slice(lo, hi)
                x = rpl_sb[:, 0:T][:, s]
                y = rpl_sb[:, T:2 * T][:, s]
                z = rpl_sb[:, 2 * T:3 * T][:, s]

                def ax(i):
                    return aux[:, i * T:(i + 1) * T][:, s]

                xx, yy, zz, s1, r2, mask, inv2, va, vb, t8 = (
                    ax(i) for i in range(10))
                eng.tensor_tensor(xx, x, x, ALU.mult)
                eng.tensor_tensor(yy, y, y, ALU.mult)
                eng.tensor_tensor(zz, z, z, ALU.mult)
                eng.tensor_tensor(s1, xx, yy, ALU.add)
                eng.tensor_tensor(r2, s1, zz, ALU.add)
                # guard r2 == 0 exactly like the reference's safe_r2
                eng.tensor_scalar(mask, r2, 0.0, None, ALU.is_equal)
                eng.tensor_tensor(mask, r2, mask, ALU.add)         # safe_r2
                nc.vector.reciprocal(inv2, mask)                   # 1/safe_r2
                nc.scalar.sqrt(va, inv2)                           # rsqrt seed
                # one Newton step: v = v*(1.5 - 0.5*safe_r2*v^2)
                eng.tensor_tensor(vb, va, va, ALU.mult)
                eng.tensor_tensor(vb, vb, mask, ALU.mult)
                eng.tensor_scalar(vb, vb, -0.5, 1.5, ALU.mult, ALU.add)
                eng.tensor_tensor(va, va, vb, ALU.mult)            # inv_r

                # chan is t-major interleaved (col = t*NCH + c) so each
                # point-tile's transpose input is one contiguous 10-col slice
                chan_v = chan[:].rearrange("p (t c) -> p c t", c=NCH)
                ch = [chan_v[:, i, :][:, s] for i in range(NCH)]
                eng.tensor_tensor(ch[0], r2, va, ALU.mult)         # radii
                eng.tensor_scalar(ch[1], r2, 0.0, 1.0, ALU.mult, ALU.add)
                eng.tensor_tensor(ch[2], y, va, ALU.mult)          # l=1
                eng.tensor_tensor(ch[3], z, va, ALU.mult)          # l=2
                eng.tensor_tensor(ch[4], x, va, ALU.mult)          # l=3
                eng.tensor_tensor(vb, x, y, ALU.mult)
                eng.tensor_tensor(ch[5], vb, inv2, ALU.mult)       # xy/r2
                eng.tensor_tensor(vb, y, z, ALU.mult)
                eng.tensor_tensor(ch[6], vb, inv2, ALU.mult)       # yz/r2
                eng.scalar_tensor_tensor(vb, zz, 3.0, r2,
                                         ALU.mult, ALU.subtract)
                eng.tensor_tensor(ch[7], vb, inv2, ALU.mult)       # (3zz-r2)/r2
                eng.tensor_tensor(vb, x, z, ALU.mult)
                eng.tensor_tensor(ch[8], vb, inv2, ALU.mult)       # xz/r2
                eng.tensor_tensor(t8, xx, yy, ALU.subtract)
                eng.tensor_tensor(ch[9], t8, inv2, ALU.mult)       # (xx-yy)/r2

            # group 0's tiles go first on the fast DVE; the rest runs on the
            # otherwise-idle Pool engine so it cannot delay group 0's chain
            compute_chan(0, TB, nc.vector)
            compute_chan(TB, T, nc.gpsimd)

            # =========================================================
            # Phase 2: main loop over 25 groups of 4 point-tiles
            # =========================================================
            with tc.tile_pool(name="tps", bufs=1, space="PSUM") as tps_pool, \
                 tc.tile_pool(name="hps", bufs=1, space="PSUM") as hps_pool, \
                 tc.tile_pool(name="rps", bufs=1, space="PSUM") as rps_pool, \
                 tc.tile_pool(name="yps", bufs=1, space="PSUM") as yps_pool, \
                 tc.tile_pool(name="kps", bufs=2, space="PSUM") as kps_pool, \
                 tc.tile_pool(name="tsb", bufs=2) as tsb_pool, \
                 tc.tile_pool(name="hsb", bufs=2) as hsb_pool, \
                 tc.tile_pool(name="bqp", bufs=2) as bqp_pool, \
                 tc.tile_pool(name="rbp", bufs=2) as rbp_pool, \
                 tc.tile_pool(name="kout", bufs=6) as kout_pool:

                def mlp_stage(g):
                    # 4 transposes: [128, 10] channel blocks -> [10, 512]
                    t0 = TB * g
                    t_ps = tps_pool.tile([NCH, TB * 128], F32, tag="t_ps")
                    for dt in range(TB):
                        nc.tensor.transpose(
                            t_ps[:, 128 * dt:128 * (dt + 1)],
                            chan[:, NCH * (t0 + dt):NCH * (t0 + dt) + NCH],
                            id_sb[:])
                    t_sb = tsb_pool.tile([NCH, TB * 128], F32R, tag="t_sb")
                    nc.scalar.copy(t_sb[:], t_ps[:])

                    # radial MLP hidden layer, whole group in one matmul
                    h_ps = hps_pool.tile([H, TB * 128], F32, tag="h_ps")
                    nc.tensor.matmul(h_ps[:], w1r_sb[:], t_sb[:],
                                     start=True, stop=True)
                    h_sb = hsb_pool.tile([H, TB * 128], F32R, tag="h_sb")
                    nc.scalar.activation(h_sb[:], h_ps[:], ACTF.Relu, bias=b1_sb[:])

                    # R (54 rows, path-repeated) and Y' (54 rows)
                    r_ps = rps_pool.tile([PL, TB * 128], F32, tag="r_ps")
                    nc.tensor.matmul(r_ps[:], w2e_sb[:], h_sb[:],
                                     start=True, stop=True)
                    y_ps = yps_pool.tile([PL, TB * 128], F32, tag="y_ps")
                    nc.tensor.matmul(y_ps[:], ey_sb[:], t_sb[:],
                                     start=True, stop=True)

                    # B = (R + b2) * Y', rounded to fp32r. The +b2 rides along
                    # the ACT psum->sbuf drain (DVE can read only one PSUM
                    # operand, so R comes via SBUF)
                    rb_sb = rbp_pool.tile([PL, TB * 128], F32, tag="rb_sb")
                    nc.scalar.activation(rb_sb[:], r_ps[:], ACTF.Identity,
                                         bias=b2_sb[:])
                    bq = bqp_pool.tile([PL, TB * 128], F32R, tag="bq")
                    nc.vector.tensor_tensor(bq[:], rb_sb[:], y_ps[:], ALU.mult)
                    return bq

                def k_stage(g, bq):
                    ndt = TB if 512 * (g + 1) <= ZC \
                        else (ZC - 512 * g + 127) // 128
                    for dt in range(ndt):
                        zt = 512 * g + 128 * dt
                        rows = min(128, ZC - zt)
                        k_ps = kps_pool.tile([128, IJ], F32, tag="k_ps")
                        for half in range(2):
                            nc.tensor.matmul(
                                k_ps[:, 512 * half:512 * (half + 1)],
                                bq[:, 128 * dt:128 * (dt + 1)],
                                m_sb[:, 512 * half:512 * (half + 1)],
                                start=True, stop=True)
                        k_sb = kout_pool.tile([128, IJ], F32, tag="k_sb")
                        # GPSIMD/Pool cannot read PSUM; split drains ACT/DVE
                        if dt % 2 == 0:
                            nc.scalar.copy(k_sb[:], k_ps[:])
                        else:
                            nc.vector.tensor_copy(k_sb[:], k_ps[:])
                        nc.sync.dma_start(out[zt:zt + rows, :], k_sb[0:rows, :])

                # software pipeline, 2 groups deep: the PE<->ACT handoff
                # chain of group g's MLP (transpose -> copy -> h -> relu ->
                # r -> +b2) spans ~6us, slightly more than one store window,
                # so contractions consume bq two groups behind production
                bqs = [mlp_stage(0), mlp_stage(1)]
                for g in range(2, NG):
                    k_stage(g - 2, bqs[g - 2])
                    bqs.append(mlp_stage(g))
                k_stage(NG - 2, bqs[NG - 2])
                k_stage(NG - 1, bqs[NG - 1])
    nc.compile()
    return nc


def _get_program():
    if "nc" not in _CACHE:
        _CACHE["nc"] = _build_program()
    return _CACHE["nc"]


def _host_prep(r, W1, b1, W2, b2, cg, ylm_mix, rf_mix, norm_coef):
    r = np.asarray(r, dtype=np.float32)
    W1 = np.asarray(W1, dtype=np.float32)
    b1 = np.asarray(b1, dtype=np.float32)
    W2 = np.asarray(W2, dtype=np.float32)
    b2 = np.asarray(b2, dtype=np.float32)
    cg = np.asarray(cg, dtype=np.float32)
    ylm_mix = np.asarray(ylm_mix, dtype=np.float32)
    rf_mix = np.asarray(rf_mix, dtype=np.float32)
    norm_coef = np.asarray(norm_coef, dtype=np.float32)

    # constant folding: M[p*9+l, ij] = sum_k rf[k,p] ylm_s[k,l] cg[k,ij] nc0[ij]
    ylm_s = ylm_mix.astype(np.float64) * YLM_SCALE[None, :]
    w_kpl = (rf_mix.astype(np.float64)[:, :, None] * ylm_s[:, None, :])
    m = w_kpl.reshape(KDIM, PL).T @ cg.astype(np.float64).reshape(KDIM, IJ)
    m *= norm_coef[:, :, 0].astype(np.float64).reshape(1, IJ)
    md = np.ascontiguousarray(m.astype(np.float32))

    w1r = np.zeros((NCH, 128), dtype=np.float32)
    w1r[0, :] = W1[0]
    eyd = np.zeros((NCH, PL), dtype=np.float32)
    for l in range(9):
        for p in range(NPATH):
            eyd[1 + l, p * 9 + l] = 1.0

    shared = {
        "w1r": w1r,
        "eyd": eyd,
        "w2e": np.ascontiguousarray(np.repeat(W2, 9, axis=1)),
        "b1c": np.ascontiguousarray(b1.reshape(H, 1)),
        "b2r": np.ascontiguousarray(np.repeat(b2, 9).reshape(PL, 1)),
        "md": md,
        "identd": np.eye(128, dtype=np.float32),
    }

    in_maps = []
    for c in range(NCORES):
        rs = r[c * ZC:(c + 1) * ZC]
        rp = np.empty((ZC_PAD, 3), dtype=np.float32)
        rp[:ZC] = rs
        rp[ZC:] = np.array([1.0, 0.0, 0.0], dtype=np.float32)
        rpl = rp.reshape(T, 128, 3).transpose(1, 2, 0).reshape(128, 3 * T)
        m = dict(shared)
        m["rpl"] = np.ascontiguousarray(rpl)
        in_maps.append(m)
    return in_maps


def _run_device(in_maps, trace=False, **kw):
    nc = _get_program()
    return run_bass_kernel_spmd(nc, in_maps, core_ids=list(range(NCORES)),
                                trace=trace, **kw)


def kernel(r, W1, b1, W2, b2, cg, ylm_mix, rf_mix, norm_coef):
    r = np.asarray(r, dtype=np.float32)
    norm_coef_f = np.asarray(norm_coef, dtype=np.float32)
    in_maps = _host_prep(r, W1, b1, W2, b2, cg, ylm_mix, rf_mix, norm_coef_f)
    res = _run_device(in_maps)
    out = np.concatenate([res.results[c]["out"] for c in range(NCORES)], axis=0)

    # points with exactly zero radius use norm_coef[..., 1] instead of [..., 0]
    x, y, z = r[:, 0], r[:, 1], r[:, 2]
    r2 = (x * x + y * y) + z * z
    zero = r2 == np.float32(0.0)
    if np.any(zero):
        scale = (norm_coef_f[:, :, 1].astype(np.float64)
                 / norm_coef_f[:, :, 0].astype(np.float64)).reshape(1, IJ)
        out[zero] = (out[zero].astype(np.float64) * scale).astype(np.float32)

    return out.reshape(Z, DO, DI)


# revision 43
# speedup vs baseline: 1.3271x; 1.0124x over previous
"""Trainium2 Bass kernel for the gnn_message_passing problem.

Math refactor: the reference computes
    kernel[z,i,j] = einsum('zk,kij->zij', Rk*Yk, cg) * nc0[i,j]
with Rk = R @ rf_mix.T (rank 6) and Yk = Y.T @ ylm_mix.T (rank 9).
Since Rk*Yk has rank<=54 over k, fold the K=1024 contraction into a
precomputed M[p*9+l, ij] = sum_k rf[k,p]*ylm_s[k,l]*cg[k,ij] * nc0[ij]
(constant folding done host-side in float64), and per point only
contract B[z, pl] = R[z,p]*Y'[z,l] against M - a k=54 matmul. This cuts
compute ~20x and makes the kernel memory-bound (410 MB output).

Distribution: data-parallel over z across 8 NeuronCores; constants
replicated. Full inputs in, full output out.

Performance notes: all matmuls run in fp32r with a moving free dim of
512 (1 cycle/row on the PE). Per 512-point group: 4 PE transposes give
the channel block in [10, 512] layout, one matmul each for the radial
hidden layer, R, and Y', a fused DVE (R+b2)*Y', then per 128-point tile
a k=54 fp32r contraction against M and a PSUM->SBUF->HBM store. The
output DMA (51.2 MB/core) is the roofline; everything else overlaps.
fp32r rounds inputs to an 11-bit mantissa (~2e-4 relative) - well
inside tolerance.
"""

import numpy as np

import concourse.bass as bass
import concourse.tile as tile
from concourse import bacc, mybir
from concourse.bass_utils import run_bass_kernel_spmd

F32 = mybir.dt.float32
F32R = mybir.dt.float32r
I32 = mybir.dt.int32
ALU = mybir.AluOpType
ACTF = mybir.ActivationFunctionType

# Problem shape (hardcoded per contract)
Z, KDIM, DO, DI, NPATH, H = 100000, 1024, 32, 32, 6, 128
IJ = DO * DI                      # 1024
NCORES = 8
ZC = Z // NCORES                  # 12500 points per core
T = 100                           # point tiles of 128 -> ZC padded to 12800
ZC_PAD = 128 * T
TB = 4                            # tiles per group
NG = T // TB                      # 25 groups
NCH = 10                          # channels: radii, ones, 8 scaled monomials
PL = NPATH * 9                    # 54 = rank of the per-point factor

# Real spherical harmonic constants (l=0,1,2), folded into M host-side
C0 = 0.28209479177387814
C1 = 0.4886025119029199
C2A = 1.0925484305920792
C2B = 0.31539156525252005
C2C = 0.5462742152960396
YLM_SCALE = np.array([C0, C1, C1, C1, C2A, C2A, C2B, C2A, C2C], dtype=np.float64)

_CACHE = {}


def _build_program():
    nc = bacc.Bacc("TRN2", target_bir_lowering=False, debug=False,
                   num_devices=NCORES)

    # ---- per-core DRAM I/O ----
    rpl = nc.dram_tensor("rpl", [128, 3 * T], F32, kind="ExternalInput").ap()
    w1r = nc.dram_tensor("w1r", [NCH, 128], F32R, kind="ExternalInput").ap()
    eyd = nc.dram_tensor("eyd", [NCH, PL], F32R, kind="ExternalInput").ap()
    w2e = nc.dram_tensor("w2e", [H, PL], F32R, kind="ExternalInput").ap()
    b1c = nc.dram_tensor("b1c", [H, 1], F32, kind="ExternalInput").ap()
    b2r = nc.dram_tensor("b2r", [PL, 1], F32, kind="ExternalInput").ap()
    md = nc.dram_tensor("md", [PL, IJ], F32R, kind="ExternalInput").ap()
    identd = nc.dram_tensor("identd", [128, 128], F32, kind="ExternalInput").ap()
    out = nc.dram_tensor("out", [ZC, IJ], F32, kind="ExternalOutput").ap()

    with tile.TileContext(nc) as tc:
        with tc.tile_pool(name="const", bufs=1) as cpool:
            # r planes first (phase 1 gates the whole pipeline); all loads on
            # the otherwise-idle SP queue, in consumption order
            rpl_sb = cpool.tile([128, 3 * T], F32)
            nc.sync.dma_start(rpl_sb[:], rpl[:])
            # ---- resident constants ----
            id_sb = cpool.tile([128, 128], F32)
            nc.sync.dma_start(id_sb[:], identd[:])
            m_sb = cpool.tile([PL, IJ], F32R)
            nc.sync.dma_start(m_sb[:], md[:])
            w1r_sb = cpool.tile([NCH, 128], F32R)
            nc.sync.dma_start(w1r_sb[:], w1r[:])
            ey_sb = cpool.tile([NCH, PL], F32R)
            nc.sync.dma_start(ey_sb[:], eyd[:])
            w2e_sb = cpool.tile([H, PL], F32R)
            nc.sync.dma_start(w2e_sb[:], w2e[:])
            b1_sb = cpool.tile([H, 1], F32)
            nc.sync.dma_start(b1_sb[:], b1c[:])
            b2_sb = cpool.tile([PL, 1], F32)
            nc.sync.dma_start(b2_sb[:], b2r[:])

            # =========================================================
            # Phase 1: per-point planes [128, T]: radii, ones, monomials
            # (emitted in two column chunks so group 0 unblocks early)
            # =========================================================
            chan = cpool.tile([128, NCH * T], F32)
            aux = cpool.tile([128, 11 * T], F32)
            # constant planes for the Pool engine, which only supports
            # TensorTensor/TensorCopy/Memset (no scalar-immediate ops, no
            # PSUM, no activation functions, no 32-bit shifts)
            cpf = cpool.tile([128, 3 * T], F32)
            nc.gpsimd.memset(cpf[:, 0:T], 1.5)
            nc.gpsimd.memset(cpf[:, T:2 * T], -0.5)
            nc.gpsimd.memset(cpf[:, 2 * T:3 * T], 1e-30)

            def compute_chan(lo, hi, eng, eng2=None):
                # eng2 (if given) takes over the l=2 channels with its own
                # scratch plane so two engines fill `chan` concurrently.
                # eng=gpsimd computes rsqrt with the bit-trick seed plus two
                # Newton steps (4.8e-6 rel) from the constant planes; the
                # DVE path uses its accurate reciprocal + ACT sqrt.
                s = slice(lo, hi)
                pool = eng is nc.gpsimd
                x = rpl_sb[:, 0:T][:, s]
                y = rpl_sb[:, T:2 * T][:, s]
                z = rpl_sb[:, 2 * T:3 * T][:, s]

                def ax(i):
                    return aux[:, i * T:(i + 1) * T][:, s]

                xx, yy, zz, s1, r2, mask, inv2, va, vb, t8 = (
                    ax(i) for i in range(10))
                eng.tensor_tensor(xx, x, x, ALU.mult)
                eng.tensor_tensor(yy, y, y, ALU.mult)
                eng.tensor_tensor(zz, z, z, ALU.mult)
                eng.tensor_tensor(s1, xx, yy, ALU.add)
                eng.tensor_tensor(r2, s1, zz, ALU.add)
                # guard r2 == 0 like the reference's safe_r2. Pool's ALU
                # only does mult/add/subtract, so that variant adds 1e-30:
                # absorbed by rounding for any real point (r2 >> 1e-23) and
                # equivalent at the origin, where every numerator is 0
                if pool:
                    eng.tensor_tensor(mask, r2, cpf[:, 2 * T:3 * T][:, s],
                                      ALU.add)
                else:
                    eng.tensor_scalar(mask, r2, 0.0, None, ALU.is_equal)
                    eng.tensor_tensor(mask, r2, mask, ALU.add)     # safe_r2
                # Pool has no reciprocal/sqrt/shift/pow: those two ops ride
                # on DVE/ACT even for the Pool chunk (a small, bounded
                # cross-engine coupling), the rest stays on `eng`
                nc.vector.reciprocal(inv2, mask)                   # 1/safe_r2
                nc.scalar.sqrt(va, inv2)                           # rsqrt seed
                # one Newton step: v = v*(1.5 - 0.5*safe_r2*v^2)
                eng.tensor_tensor(vb, va, va, ALU.mult)
                eng.tensor_tensor(vb, vb, mask, ALU.mult)
                if pool:
                    eng.tensor_tensor(vb, vb, cpf[:, T:2 * T][:, s], ALU.mult)
                    eng.tensor_tensor(vb, vb, cpf[:, 0:T][:, s], ALU.add)
                else:
                    eng.tensor_scalar(vb, vb, -0.5, 1.5, ALU.mult, ALU.add)
                eng.tensor_tensor(va, va, vb, ALU.mult)            # inv_r

                # chan is t-major interleaved (col = t*NCH + c) so each
                # point-tile's transpose input is one contiguous 10-col slice
                chan_v = chan[:].rearrange("p (t c) -> p c t", c=NCH)
                ch = [chan_v[:, i, :][:, s] for i in range(NCH)]
                eng.tensor_tensor(ch[0], r2, va, ALU.mult)         # radii
                eng.memset(ch[1], 1.0)                             # ones
                eng.tensor_tensor(ch[2], y, va, ALU.mult)          # l=1
                eng.tensor_tensor(ch[3], z, va, ALU.mult)          # l=2
                eng.tensor_tensor(ch[4], x, va, ALU.mult)          # l=3
                e2 = eng2 or eng
                w = aux[:, 10 * T:11 * T][:, s] if eng2 else vb
                e2.tensor_tensor(w, x, y, ALU.mult)
                e2.tensor_tensor(ch[5], w, inv2, ALU.mult)         # xy/r2
                e2.tensor_tensor(w, y, z, ALU.mult)
                e2.tensor_tensor(ch[6], w, inv2, ALU.mult)         # yz/r2
                e2.tensor_tensor(w, zz, zz, ALU.add)
                e2.tensor_tensor(w, w, zz, ALU.add)
                e2.tensor_tensor(w, w, r2, ALU.subtract)
                e2.tensor_tensor(ch[7], w, inv2, ALU.mult)         # (3zz-r2)/r2
                e2.tensor_tensor(w, x, z, ALU.mult)
                e2.tensor_tensor(ch[8], w, inv2, ALU.mult)         # xz/r2
                e2.tensor_tensor(t8, xx, yy, ALU.subtract)
                e2.tensor_tensor(ch[9], t8, inv2, ALU.mult)        # (xx-yy)/r2

            # the first four groups' tiles (the pipeline-fill window) go
            # first on DVE - the serial chain cost is per-op, not per-
            # column; the rest runs 100% on Pool (emitted after the first
            # two MLP stages below), so no early-group transpose can get
            # scheduled on the in-order PE behind the bulk Pool chunk
            compute_chan(0, 3 * TB, nc.vector)

            # =========================================================
            # Phase 2: main loop over point-tile groups (the first two are
            # half-sized so the pipeline-fill latency is shorter)
            # =========================================================
            with tc.tile_pool(name="tps", bufs=1, space="PSUM") as tps_pool, \
                 tc.tile_pool(name="hps", bufs=1, space="PSUM") as hps_pool, \
                 tc.tile_pool(name="rps", bufs=1, space="PSUM") as rps_pool, \
                 tc.tile_pool(name="yps", bufs=1, space="PSUM") as yps_pool, \
                 tc.tile_pool(name="kps", bufs=2, space="PSUM") as kps_pool, \
                 tc.tile_pool(name="tsb", bufs=2) as tsb_pool, \
                 tc.tile_pool(name="hsb", bufs=2) as hsb_pool, \
                 tc.tile_pool(name="bqp", bufs=2) as bqp_pool, \
                 tc.tile_pool(name="rbp", bufs=2) as rbp_pool, \
                 tc.tile_pool(name="kout", bufs=6) as kout_pool:

                def mlp_stage(t0, nt):
                    # nt transposes: [128, 10] channel blocks -> [10, nt*128]
                    n = nt * 128
                    t_ps = tps_pool.tile([NCH, TB * 128], F32, tag="t_ps")
                    for dt in range(nt):
                        nc.tensor.transpose(
                            t_ps[:, 128 * dt:128 * (dt + 1)],
                            chan[:, NCH * (t0 + dt):NCH * (t0 + dt) + NCH],
                            id_sb[:])
                    t_sb = tsb_pool.tile([NCH, TB * 128], F32R, tag="t_sb")
                    nc.scalar.copy(t_sb[:, :n], t_ps[:, :n])

                    # radial MLP hidden layer, whole group in one matmul
                    h_ps = hps_pool.tile([H, TB * 128], F32, tag="h_ps")
                    nc.tensor.matmul(h_ps[:, :n], w1r_sb[:], t_sb[:, :n],
                                     start=True, stop=True)
                    h_sb = hsb_pool.tile([H, TB * 128], F32R, tag="h_sb")
                    nc.scalar.activation(h_sb[:, :n], h_ps[:, :n], ACTF.Relu,
                                         bias=b1_sb[:])

                    # R (54 rows, path-repeated) and Y' (54 rows)
                    r_ps = rps_pool.tile([PL, TB * 128], F32, tag="r_ps")
                    nc.tensor.matmul(r_ps[:, :n], w2e_sb[:], h_sb[:, :n],
                                     start=True, stop=True)
                    y_ps = yps_pool.tile([PL, TB * 128], F32, tag="y_ps")
                    nc.tensor.matmul(y_ps[:, :n], ey_sb[:], t_sb[:, :n],
                                     start=True, stop=True)

                    # B = (R + b2) * Y', rounded to fp32r. The +b2 rides along
                    # the ACT psum->sbuf drain (DVE can read only one PSUM
                    # operand, so R comes via SBUF)
                    rb_sb = rbp_pool.tile([PL, TB * 128], F32, tag="rb_sb")
                    nc.scalar.activation(rb_sb[:, :n], r_ps[:, :n],
                                         ACTF.Identity, bias=b2_sb[:])
                    bq = bqp_pool.tile([PL, TB * 128], F32R, tag="bq")
                    nc.vector.tensor_tensor(bq[:, :n], rb_sb[:, :n],
                                            y_ps[:, :n], ALU.mult)
                    return bq

                def k_stage(t0, nt, bq):
                    for dt in range(nt):
                        zt = 128 * (t0 + dt)
                        if zt >= ZC:
                            break
                        rows = min(128, ZC - zt)
                        k_ps = kps_pool.tile([128, IJ], F32, tag="k_ps")
                        for half in range(2):
                            nc.tensor.matmul(
                                k_ps[:, 512 * half:512 * (half + 1)],
                                bq[:, 128 * dt:128 * (dt + 1)],
                                m_sb[:, 512 * half:512 * (half + 1)],
                                start=True, stop=True)
                        k_sb = kout_pool.tile([128, IJ], F32, tag="k_sb")
                        # GPSIMD/Pool cannot read PSUM; split drains ACT/DVE
                        if dt % 2 == 0:
                            nc.scalar.copy(k_sb[:], k_ps[:])
                        else:
                            nc.vector.tensor_copy(k_sb[:], k_ps[:])
                        nc.sync.dma_start(out[zt:zt + rows, :], k_sb[0:rows, :])

                # software pipeline, 2 groups deep: the PE<->ACT handoff
                # chain of group g's MLP (transpose -> copy -> h -> relu ->
                # r -> +b2) spans ~6us, slightly more than one store window,
                # so contractions consume bq two groups behind production
                groups = [(0, 2), (2, 2)] + [(t0, TB) for t0 in range(TB, T, TB)]
                bqs = [mlp_stage(*groups[0]), mlp_stage(*groups[1])]
                compute_chan(3 * TB, T, nc.gpsimd)
                for i in range(2, len(groups)):
                    if i <= 3:
                        # pin the pipeline-fill groups ahead of everything
                        # in the list scheduler's priority heap
                        with tc.high_priority():
                            k_stage(*groups[i - 2], bqs[i - 2])
                    else:
                        k_stage(*groups[i - 2], bqs[i - 2])
                    bqs.append(mlp_stage(*groups[i]))
                k_stage(*groups[-2], bqs[-2])
                k_stage(*groups[-1], bqs[-1])
    nc.compile()
    return nc


def _get_program():
    if "nc" not in _CACHE:
        _CACHE["nc"] = _build_program()
    return _CACHE["nc"]


def _host_prep(r, W1, b1, W2, b2, cg, ylm_mix, rf_mix, norm_coef):
    r = np.asarray(r, dtype=np.float32)
    W1 = np.asarray(W1, dtype=np.float32)
    b1 = np.asarray(b1, dtype=np.float32)
    W2 = np.asarray(W2, dtype=np.float32)
    b2 = np.asarray(b2, dtype=np.float32)
    cg = np.asarray(cg, dtype=np.float32)
    ylm_mix = np.asarray(ylm_mix, dtype=np.float32)
    rf_mix = np.asarray(rf_mix, dtype=np.float32)
    norm_coef = np.asarray(norm_coef, dtype=np.float32)

    # constant folding: M[p*9+l, ij] = sum_k rf[k,p] ylm_s[k,l] cg[k,ij] nc0[ij]
    ylm_s = ylm_mix.astype(np.float64) * YLM_SCALE[None, :]
    w_kpl = (rf_mix.astype(np.float64)[:, :, None] * ylm_s[:, None, :])
    m = w_kpl.reshape(KDIM, PL).T @ cg.astype(np.float64).reshape(KDIM, IJ)
    m *= norm_coef[:, :, 0].astype(np.float64).reshape(1, IJ)
    md = np.ascontiguousarray(m.astype(np.float32))

    w1r = np.zeros((NCH, 128), dtype=np.float32)
    w1r[0, :] = W1[0]
    eyd = np.zeros((NCH, PL), dtype=np.float32)
    for l in range(9):
        for p in range(NPATH):
            eyd[1 + l, p * 9 + l] = 1.0

    shared = {
        "w1r": w1r,
        "eyd": eyd,
        "w2e": np.ascontiguousarray(np.repeat(W2, 9, axis=1)),
        "b1c": np.ascontiguousarray(b1.reshape(H, 1)),
        "b2r": np.ascontiguousarray(np.repeat(b2, 9).reshape(PL, 1)),
        "md": md,
        "identd": np.eye(128, dtype=np.float32),
    }

    in_maps = []
    for c in range(NCORES):
        rs = r[c * ZC:(c + 1) * ZC]
        rp = np.empty((ZC_PAD, 3), dtype=np.float32)
        rp[:ZC] = rs
        rp[ZC:] = np.array([1.0, 0.0, 0.0], dtype=np.float32)
        rpl = rp.reshape(T, 128, 3).transpose(1, 2, 0).reshape(128, 3 * T)
        m = dict(shared)
        m["rpl"] = np.ascontiguousarray(rpl)
        in_maps.append(m)
    return in_maps


def _run_device(in_maps, trace=False, **kw):
    nc = _get_program()
    return run_bass_kernel_spmd(nc, in_maps, core_ids=list(range(NCORES)),
                                trace=trace, **kw)


def kernel(r, W1, b1, W2, b2, cg, ylm_mix, rf_mix, norm_coef):
    r = np.asarray(r, dtype=np.float32)
    norm_coef_f = np.asarray(norm_coef, dtype=np.float32)
    in_maps = _host_prep(r, W1, b1, W2, b2, cg, ylm_mix, rf_mix, norm_coef_f)
    res = _run_device(in_maps)
    out = np.concatenate([res.results[c]["out"] for c in range(NCORES)], axis=0)

    # points with exactly zero radius use norm_coef[..., 1] instead of [..., 0]
    x, y, z = r[:, 0], r[:, 1], r[:, 2]
    r2 = (x * x + y * y) + z * z
    zero = r2 == np.float32(0.0)
    if np.any(zero):
        scale = (norm_coef_f[:, :, 1].astype(np.float64)
                 / norm_coef_f[:, :, 0].astype(np.float64)).reshape(1, IJ)
        out[zero] = (out[zero].astype(np.float64) * scale).astype(np.float32)

    return out.reshape(Z, DO, DI)


# revision 46
# speedup vs baseline: 1.3410x; 1.0105x over previous
"""Trainium2 Bass kernel for the gnn_message_passing problem.

Math refactor: the reference computes
    kernel[z,i,j] = einsum('zk,kij->zij', Rk*Yk, cg) * nc0[i,j]
with Rk = R @ rf_mix.T (rank 6) and Yk = Y.T @ ylm_mix.T (rank 9).
Since Rk*Yk has rank<=54 over k, fold the K=1024 contraction into a
precomputed M[p*9+l, ij] = sum_k rf[k,p]*ylm_s[k,l]*cg[k,ij] * nc0[ij]
(constant folding done host-side in float64), and per point only
contract B[z, pl] = R[z,p]*Y'[z,l] against M - a k=54 matmul. This cuts
compute ~20x and makes the kernel memory-bound (410 MB output).

Distribution: data-parallel over z across 8 NeuronCores; constants
replicated. Full inputs in, full output out.

Performance notes: all matmuls run in fp32r with a moving free dim of
512 (1 cycle/row on the PE). Per 512-point group: 4 PE transposes give
the channel block in [10, 512] layout, one matmul each for the radial
hidden layer, R, and Y', a fused DVE (R+b2)*Y', then per 128-point tile
a k=54 fp32r contraction against M and a PSUM->SBUF->HBM store. The
output DMA (51.2 MB/core) is the roofline; everything else overlaps.
fp32r rounds inputs to an 11-bit mantissa (~2e-4 relative) - well
inside tolerance.
"""

import numpy as np

import concourse.bass as bass
import concourse.tile as tile
from concourse import bacc, mybir
from concourse.bass_utils import run_bass_kernel_spmd

F32 = mybir.dt.float32
F32R = mybir.dt.float32r
I32 = mybir.dt.int32
ALU = mybir.AluOpType
ACTF = mybir.ActivationFunctionType

# Problem shape (hardcoded per contract)
Z, KDIM, DO, DI, NPATH, H = 100000, 1024, 32, 32, 6, 128
IJ = DO * DI                      # 1024
NCORES = 8
ZC = Z // NCORES                  # 12500 points per core
T = 100                           # point tiles of 128 -> ZC padded to 12800
ZC_PAD = 128 * T
TB = 4                            # tiles per group
NG = T // TB                      # 25 groups
NCH = 10                          # channels: radii, ones, 8 scaled monomials
PL = NPATH * 9                    # 54 = rank of the per-point factor

# Real spherical harmonic constants (l=0,1,2), folded into M host-side
C0 = 0.28209479177387814
C1 = 0.4886025119029199
C2A = 1.0925484305920792
C2B = 0.31539156525252005
C2C = 0.5462742152960396
YLM_SCALE = np.array([C0, C1, C1, C1, C2A, C2A, C2B, C2A, C2C], dtype=np.float64)

_CACHE = {}


def _build_program():
    nc = bacc.Bacc("TRN2", target_bir_lowering=False, debug=False,
                   num_devices=NCORES)

    # ---- per-core DRAM I/O ----
    rpl = nc.dram_tensor("rpl", [128, 3 * T], F32, kind="ExternalInput").ap()
    w1r = nc.dram_tensor("w1r", [NCH, 128], F32R, kind="ExternalInput").ap()
    eyd = nc.dram_tensor("eyd", [NCH, PL], F32R, kind="ExternalInput").ap()
    w2e = nc.dram_tensor("w2e", [H, PL], F32R, kind="ExternalInput").ap()
    b1c = nc.dram_tensor("b1c", [H, 1], F32, kind="ExternalInput").ap()
    b2r = nc.dram_tensor("b2r", [PL, 1], F32, kind="ExternalInput").ap()
    md = nc.dram_tensor("md", [PL, IJ], F32R, kind="ExternalInput").ap()
    identd = nc.dram_tensor("identd", [128, 128], F32, kind="ExternalInput").ap()
    out = nc.dram_tensor("out", [ZC, IJ], F32, kind="ExternalOutput").ap()

    with tile.TileContext(nc) as tc:
        with tc.tile_pool(name="const", bufs=1) as cpool:
            # r planes first (phase 1 gates the whole pipeline); all loads on
            # the otherwise-idle SP queue, in consumption order
            rpl_sb = cpool.tile([128, 3 * T], F32)
            nc.sync.dma_start(rpl_sb[:], rpl[:])
            # ---- resident constants ----
            id_sb = cpool.tile([128, 128], F32)
            nc.sync.dma_start(id_sb[:], identd[:])
            m_sb = cpool.tile([PL, IJ], F32R)
            nc.sync.dma_start(m_sb[:], md[:])
            w1r_sb = cpool.tile([NCH, 128], F32R)
            nc.sync.dma_start(w1r_sb[:], w1r[:])
            ey_sb = cpool.tile([NCH, PL], F32R)
            nc.sync.dma_start(ey_sb[:], eyd[:])
            w2e_sb = cpool.tile([H, PL], F32R)
            nc.sync.dma_start(w2e_sb[:], w2e[:])
            b1_sb = cpool.tile([H, 1], F32)
            nc.sync.dma_start(b1_sb[:], b1c[:])
            b2_sb = cpool.tile([PL, 1], F32)
            nc.sync.dma_start(b2_sb[:], b2r[:])

            # =========================================================
            # Phase 1: per-point planes [128, T]: radii, ones, monomials
            # (emitted in two column chunks so group 0 unblocks early)
            # =========================================================
            chan = cpool.tile([128, NCH * T], F32)
            aux = cpool.tile([128, 11 * T], F32)
            # constant planes for the Pool engine, which only supports
            # TensorTensor/TensorCopy/Memset (no scalar-immediate ops, no
            # PSUM, no activation functions, no 32-bit shifts)
            cpf = cpool.tile([128, 3 * T], F32)
            nc.gpsimd.memset(cpf[:, 2 * T:3 * T], 1e-30)

            def compute_chan(lo, hi, eng, eng2=None):
                # eng2 (if given) takes over the l=2 channels with its own
                # scratch plane so two engines fill `chan` concurrently.
                # eng=gpsimd computes rsqrt with the bit-trick seed plus two
                # Newton steps (4.8e-6 rel) from the constant planes; the
                # DVE path uses its accurate reciprocal + ACT sqrt.
                s = slice(lo, hi)
                pool = eng is nc.gpsimd
                x = rpl_sb[:, 0:T][:, s]
                y = rpl_sb[:, T:2 * T][:, s]
                z = rpl_sb[:, 2 * T:3 * T][:, s]

                def ax(i):
                    return aux[:, i * T:(i + 1) * T][:, s]

                xx, yy, zz, s1, r2, mask, inv2, va, vb, t8 = (
                    ax(i) for i in range(10))
                eng.tensor_tensor(xx, x, x, ALU.mult)
                eng.tensor_tensor(yy, y, y, ALU.mult)
                eng.tensor_tensor(zz, z, z, ALU.mult)
                eng.tensor_tensor(s1, xx, yy, ALU.add)
                eng.tensor_tensor(r2, s1, zz, ALU.add)
                # guard r2 == 0 like the reference's safe_r2 by adding
                # 1e-30: absorbed by rounding for any real point (r2 >>
                # 1e-23) and equivalent at the origin, where every
                # monomial numerator is exactly 0 (Pool's ALU only does
                # mult/add/subtract, so no is_equal masking there)
                if pool:
                    eng.tensor_tensor(mask, r2, cpf[:, 2 * T:3 * T][:, s],
                                      ALU.add)
                else:
                    eng.tensor_scalar(mask, r2, 1e-30, None, ALU.add)
                # Pool has no reciprocal/sqrt/shift/pow: these two ops ride
                # on DVE/ACT even for the Pool chunk (a small, bounded
                # cross-engine coupling), the rest stays on `eng`.
                # DVE reciprocal is exact, ACT sqrt ~7e-6 - well under the
                # fp32r rounding (2.4e-4) that dominates the error budget,
                # so no Newton polish is needed.
                nc.vector.reciprocal(inv2, mask)                   # 1/safe_r2
                nc.scalar.sqrt(va, inv2)                           # inv_r

                # chan is t-major interleaved (col = t*NCH + c) so each
                # point-tile's transpose input is one contiguous 10-col slice
                chan_v = chan[:].rearrange("p (t c) -> p c t", c=NCH)
                ch = [chan_v[:, i, :][:, s] for i in range(NCH)]
                eng.tensor_tensor(ch[0], r2, va, ALU.mult)         # radii
                eng.memset(ch[1], 1.0)                             # ones
                eng.tensor_tensor(ch[2], y, va, ALU.mult)          # l=1
                eng.tensor_tensor(ch[3], z, va, ALU.mult)          # l=2
                eng.tensor_tensor(ch[4], x, va, ALU.mult)          # l=3
                e2 = eng2 or eng
                w = aux[:, 10 * T:11 * T][:, s] if eng2 else vb
                e2.tensor_tensor(w, x, y, ALU.mult)
                e2.tensor_tensor(ch[5], w, inv2, ALU.mult)         # xy/r2
                e2.tensor_tensor(w, y, z, ALU.mult)
                e2.tensor_tensor(ch[6], w, inv2, ALU.mult)         # yz/r2
                if pool:
                    e2.tensor_tensor(w, zz, zz, ALU.add)
                    e2.tensor_tensor(w, w, zz, ALU.add)
                    e2.tensor_tensor(w, w, r2, ALU.subtract)
                else:
                    e2.scalar_tensor_tensor(w, zz, 3.0, r2,
                                            ALU.mult, ALU.subtract)
                e2.tensor_tensor(ch[7], w, inv2, ALU.mult)         # (3zz-r2)/r2
                e2.tensor_tensor(w, x, z, ALU.mult)
                e2.tensor_tensor(ch[8], w, inv2, ALU.mult)         # xz/r2
                e2.tensor_tensor(t8, xx, yy, ALU.subtract)
                e2.tensor_tensor(ch[9], t8, inv2, ALU.mult)        # (xx-yy)/r2

            # the first four groups' tiles (the pipeline-fill window) go
            # first on DVE - the serial chain cost is per-op, not per-
            # column; the rest runs 100% on Pool (emitted after the first
            # two MLP stages below), so no early-group transpose can get
            # scheduled on the in-order PE behind the bulk Pool chunk
            compute_chan(0, 3 * TB, nc.vector)

            # =========================================================
            # Phase 2: main loop over point-tile groups (the first two are
            # half-sized so the pipeline-fill latency is shorter)
            # =========================================================
            with tc.tile_pool(name="tps", bufs=1, space="PSUM") as tps_pool, \
                 tc.tile_pool(name="hps", bufs=1, space="PSUM") as hps_pool, \
                 tc.tile_pool(name="rps", bufs=1, space="PSUM") as rps_pool, \
                 tc.tile_pool(name="yps", bufs=1, space="PSUM") as yps_pool, \
                 tc.tile_pool(name="kps", bufs=2, space="PSUM") as kps_pool, \
                 tc.tile_pool(name="tsb", bufs=2) as tsb_pool, \
                 tc.tile_pool(name="hsb", bufs=2) as hsb_pool, \
                 tc.tile_pool(name="bqp", bufs=2) as bqp_pool, \
                 tc.tile_pool(name="rbp", bufs=2) as rbp_pool, \
                 tc.tile_pool(name="kout", bufs=6) as kout_pool:

                def mlp_stage(t0, nt):
                    # nt transposes: [128, 10] channel blocks -> [10, nt*128]
                    n = nt * 128
                    t_ps = tps_pool.tile([NCH, TB * 128], F32, tag="t_ps")
                    for dt in range(nt):
                        nc.tensor.transpose(
                            t_ps[:, 128 * dt:128 * (dt + 1)],
                            chan[:, NCH * (t0 + dt):NCH * (t0 + dt) + NCH],
                            id_sb[:])
                    t_sb = tsb_pool.tile([NCH, TB * 128], F32R, tag="t_sb")
                    nc.scalar.copy(t_sb[:, :n], t_ps[:, :n])

                    # radial MLP hidden layer, whole group in one matmul
                    h_ps = hps_pool.tile([H, TB * 128], F32, tag="h_ps")
                    nc.tensor.matmul(h_ps[:, :n], w1r_sb[:], t_sb[:, :n],
                                     start=True, stop=True)
                    h_sb = hsb_pool.tile([H, TB * 128], F32R, tag="h_sb")
                    nc.scalar.activation(h_sb[:, :n], h_ps[:, :n], ACTF.Relu,
                                         bias=b1_sb[:])

                    # R (54 rows, path-repeated) and Y' (54 rows)
                    r_ps = rps_pool.tile([PL, TB * 128], F32, tag="r_ps")
                    nc.tensor.matmul(r_ps[:, :n], w2e_sb[:], h_sb[:, :n],
                                     start=True, stop=True)
                    y_ps = yps_pool.tile([PL, TB * 128], F32, tag="y_ps")
                    nc.tensor.matmul(y_ps[:, :n], ey_sb[:], t_sb[:, :n],
                                     start=True, stop=True)

                    # B = (R + b2) * Y', rounded to fp32r. The +b2 rides along
                    # the ACT psum->sbuf drain (DVE can read only one PSUM
                    # operand, so R comes via SBUF)
                    rb_sb = rbp_pool.tile([PL, TB * 128], F32, tag="rb_sb")
                    nc.scalar.activation(rb_sb[:, :n], r_ps[:, :n],
                                         ACTF.Identity, bias=b2_sb[:])
                    bq = bqp_pool.tile([PL, TB * 128], F32R, tag="bq")
                    nc.vector.tensor_tensor(bq[:, :n], rb_sb[:, :n],
                                            y_ps[:, :n], ALU.mult)
                    return bq

                def k_stage(t0, nt, bq):
                    for dt in range(nt):
                        zt = 128 * (t0 + dt)
                        if zt >= ZC:
                            break
                        rows = min(128, ZC - zt)
                        k_ps = kps_pool.tile([128, IJ], F32, tag="k_ps")
                        for half in range(2):
                            nc.tensor.matmul(
                                k_ps[:, 512 * half:512 * (half + 1)],
                                bq[:, 128 * dt:128 * (dt + 1)],
                                m_sb[:, 512 * half:512 * (half + 1)],
                                start=True, stop=True)
                        k_sb = kout_pool.tile([128, IJ], F32, tag="k_sb")
                        # GPSIMD/Pool cannot read PSUM; split drains ACT/DVE
                        if dt % 2 == 0:
                            nc.scalar.copy(k_sb[:], k_ps[:])
                        else:
                            nc.vector.tensor_copy(k_sb[:], k_ps[:])
                        nc.sync.dma_start(out[zt:zt + rows, :], k_sb[0:rows, :])

                # software pipeline, 2 groups deep: the PE<->ACT handoff
                # chain of group g's MLP (transpose -> copy -> h -> relu ->
                # r -> +b2) spans ~6us, slightly more than one store window,
                # so contractions consume bq two groups behind production
                groups = [(0, 2), (2, 2)] + [(t0, TB) for t0 in range(TB, T, TB)]
                bqs = [mlp_stage(*groups[0]), mlp_stage(*groups[1])]
                compute_chan(3 * TB, T, nc.gpsimd)
                for i in range(2, len(groups)):
                    if i <= 3:
                        # pin the pipeline-fill groups ahead of everything
                        # in the list scheduler's priority heap
                        with tc.high_priority():
                            k_stage(*groups[i - 2], bqs[i - 2])
                    else:
                        k_stage(*groups[i - 2], bqs[i - 2])
                    bqs.append(mlp_stage(*groups[i]))
                k_stage(*groups[-2], bqs[-2])
                k_stage(*groups[-1], bqs[-1])
    nc.compile()
    return nc


def _get_program():
    if "nc" not in _CACHE:
        _CACHE["nc"] = _build_program()
    return _CACHE["nc"]


def _host_prep(r, W1, b1, W2, b2, cg, ylm_mix, rf_mix, norm_coef):
    r = np.asarray(r, dtype=np.float32)
    W1 = np.asarray(W1, dtype=np.float32)
    b1 = np.asarray(b1, dtype=np.float32)
    W2 = np.asarray(W2, dtype=np.float32)
    b2 = np.asarray(b2, dtype=np.float32)
    cg = np.asarray(cg, dtype=np.float32)
    ylm_mix = np.asarray(ylm_mix, dtype=np.float32)
    rf_mix = np.asarray(rf_mix, dtype=np.float32)
    norm_coef = np.asarray(norm_coef, dtype=np.float32)

    # constant folding: M[p*9+l, ij] = sum_k rf[k,p] ylm_s[k,l] cg[k,ij] nc0[ij]
    ylm_s = ylm_mix.astype(np.float64) * YLM_SCALE[None, :]
    w_kpl = (rf_mix.astype(np.float64)[:, :, None] * ylm_s[:, None, :])
    m = w_kpl.reshape(KDIM, PL).T @ cg.astype(np.float64).reshape(KDIM, IJ)
    m *= norm_coef[:, :, 0].astype(np.float64).reshape(1, IJ)
    md = np.ascontiguousarray(m.astype(np.float32))

    w1r = np.zeros((NCH, 128), dtype=np.float32)
    w1r[0, :] = W1[0]
    eyd = np.zeros((NCH, PL), dtype=np.float32)
    for l in range(9):
        for p in range(NPATH):
            eyd[1 + l, p * 9 + l] = 1.0

    shared = {
        "w1r": w1r,
        "eyd": eyd,
        "w2e": np.ascontiguousarray(np.repeat(W2, 9, axis=1)),
        "b1c": np.ascontiguousarray(b1.reshape(H, 1)),
        "b2r": np.ascontiguousarray(np.repeat(b2, 9).reshape(PL, 1)),
        "md": md,
        "identd": np.eye(128, dtype=np.float32),
    }

    in_maps = []
    for c in range(NCORES):
        rs = r[c * ZC:(c + 1) * ZC]
        rp = np.empty((ZC_PAD, 3), dtype=np.float32)
        rp[:ZC] = rs
        rp[ZC:] = np.array([1.0, 0.0, 0.0], dtype=np.float32)
        rpl = rp.reshape(T, 128, 3).transpose(1, 2, 0).reshape(128, 3 * T)
        m = dict(shared)
        m["rpl"] = np.ascontiguousarray(rpl)
        in_maps.append(m)
    return in_maps


def _run_device(in_maps, trace=False, **kw):
    nc = _get_program()
    return run_bass_kernel_spmd(nc, in_maps, core_ids=list(range(NCORES)),
                                trace=trace, **kw)


def kernel(r, W1, b1, W2, b2, cg, ylm_mix, rf_mix, norm_coef):
    r = np.asarray(r, dtype=np.float32)
    norm_coef_f = np.asarray(norm_coef, dtype=np.float32)
    in_maps = _host_prep(r, W1, b1, W2, b2, cg, ylm_mix, rf_mix, norm_coef_f)
    res = _run_device(in_maps)
    out = np.concatenate([res.results[c]["out"] for c in range(NCORES)], axis=0)

    # points with exactly zero radius use norm_coef[..., 1] instead of [..., 0]
    x, y, z = r[:, 0], r[:, 1], r[:, 2]
    r2 = (x * x + y * y) + z * z
    zero = r2 == np.float32(0.0)
    if np.any(zero):
        scale = (norm_coef_f[:, :, 1].astype(np.float64)
                 / norm_coef_f[:, :, 0].astype(np.float64)).reshape(1, IJ)
        out[zero] = (out[zero].astype(np.float64) * scale).astype(np.float32)

    return out.reshape(Z, DO, DI)


# revision 50
# speedup vs baseline: 1.3496x; 1.0064x over previous
"""Trainium2 Bass kernel for the gnn_message_passing problem.

Math refactor: the reference computes
    kernel[z,i,j] = einsum('zk,kij->zij', Rk*Yk, cg) * nc0[i,j]
with Rk = R @ rf_mix.T (rank 6) and Yk = Y.T @ ylm_mix.T (rank 9).
Since Rk*Yk has rank<=54 over k, fold the K=1024 contraction into a
precomputed M[p*9+l, ij] = sum_k rf[k,p]*ylm_s[k,l]*cg[k,ij] * nc0[ij]
(constant folding done host-side in float64), and per point only
contract B[z, pl] = R[z,p]*Y'[z,l] against M - a k=54 matmul. This cuts
compute ~20x and makes the kernel memory-bound (410 MB output).

Distribution: data-parallel over z across 8 NeuronCores; constants
replicated. Full inputs in, full output out.

Performance notes: all matmuls run in fp32r with a moving free dim of
256+ (1 cycle/row on the PE). Per point group: PE transposes give the
channel block in [10, n*128] layout, one matmul each for the radial
hidden layer, R (with b2 folded into the ACT psum drain), and Y', a
DVE multiply into B, then per 128-point tile a k=54 fp32r contraction
against M and a PSUM->SBUF->HBM store. The output DMA (51.2 MB/core,
~142us at 360 GB/s) is the roofline; the loop is software-pipelined
two groups deep so every other engine hides behind it, and the
pipeline-fill window (first four groups, half-sized first two) gets
its channel planes from DVE while Pool covers the rest. fp32r rounds
inputs to an 11-bit mantissa (~2e-4 relative) - well inside tolerance.
"""

import numpy as np

import concourse.bass as bass
import concourse.tile as tile
from concourse import bacc, mybir
from concourse.bass_utils import run_bass_kernel_spmd

F32 = mybir.dt.float32
F32R = mybir.dt.float32r
I32 = mybir.dt.int32
ALU = mybir.AluOpType
ACTF = mybir.ActivationFunctionType

# Problem shape (hardcoded per contract)
Z, KDIM, DO, DI, NPATH, H = 100000, 1024, 32, 32, 6, 128
IJ = DO * DI                      # 1024
NCORES = 8
ZC = Z // NCORES                  # 12500 points per core
T = 100                           # point tiles of 128 -> ZC padded to 12800
ZC_PAD = 128 * T
TB = 4                            # tiles per group
NG = T // TB                      # 25 groups
NCH = 10                          # channels: radii, ones, 8 scaled monomials
PL = NPATH * 9                    # 54 = rank of the per-point factor

# Real spherical harmonic constants (l=0,1,2), folded into M host-side
C0 = 0.28209479177387814
C1 = 0.4886025119029199
C2A = 1.0925484305920792
C2B = 0.31539156525252005
C2C = 0.5462742152960396
YLM_SCALE = np.array([C0, C1, C1, C1, C2A, C2A, C2B, C2A, C2C], dtype=np.float64)

_CACHE = {}


def _build_program():
    nc = bacc.Bacc("TRN2", target_bir_lowering=False, debug=False,
                   num_devices=NCORES)

    # ---- per-core DRAM I/O ----
    rpl = nc.dram_tensor("rpl", [128, 3 * T], F32, kind="ExternalInput").ap()
    w1r = nc.dram_tensor("w1r", [NCH, 128], F32R, kind="ExternalInput").ap()
    eyd = nc.dram_tensor("eyd", [NCH, PL], F32R, kind="ExternalInput").ap()
    w2e = nc.dram_tensor("w2e", [H, PL], F32R, kind="ExternalInput").ap()
    b1c = nc.dram_tensor("b1c", [H, 1], F32, kind="ExternalInput").ap()
    b2r = nc.dram_tensor("b2r", [PL, 1], F32, kind="ExternalInput").ap()
    md = nc.dram_tensor("md", [PL, IJ], F32R, kind="ExternalInput").ap()
    identd = nc.dram_tensor("identd", [128, 128], F32, kind="ExternalInput").ap()
    out = nc.dram_tensor("out", [ZC, IJ], F32, kind="ExternalOutput").ap()

    with tile.TileContext(nc) as tc:
        with tc.tile_pool(name="const", bufs=1) as cpool:
            # r planes first (phase 1 gates the whole pipeline); all loads on
            # the otherwise-idle SP queue, in consumption order
            rpl_sb = cpool.tile([128, 3 * T], F32)
            nc.sync.dma_start(rpl_sb[:], rpl[:])
            # ---- resident constants ----
            id_sb = cpool.tile([128, 128], F32)
            nc.sync.dma_start(id_sb[:], identd[:])
            m_sb = cpool.tile([PL, IJ], F32R)
            nc.sync.dma_start(m_sb[:], md[:])
            w1r_sb = cpool.tile([NCH, 128], F32R)
            nc.sync.dma_start(w1r_sb[:], w1r[:])
            ey_sb = cpool.tile([NCH, PL], F32R)
            nc.sync.dma_start(ey_sb[:], eyd[:])
            w2e_sb = cpool.tile([H, PL], F32R)
            nc.sync.dma_start(w2e_sb[:], w2e[:])
            b1_sb = cpool.tile([H, 1], F32)
            nc.sync.dma_start(b1_sb[:], b1c[:])
            b2_sb = cpool.tile([PL, 1], F32)
            nc.sync.dma_start(b2_sb[:], b2r[:])

            # =========================================================
            # Phase 1: per-point planes [128, T]: radii, ones, monomials
            # (emitted in two column chunks so group 0 unblocks early)
            # =========================================================
            chan = cpool.tile([128, NCH * T], F32)
            aux = cpool.tile([128, 11 * T], F32)
            # constant planes for the Pool engine, which only supports
            # TensorTensor/TensorCopy/Memset (no scalar-immediate ops, no
            # PSUM, no activation functions, no 32-bit shifts)
            cpf = cpool.tile([128, 3 * T], F32)
            nc.gpsimd.memset(cpf[:, 2 * T:3 * T], 1e-30)

            def compute_chan(lo, hi, eng, eng2=None):
                # eng2 (if given) takes over the l=2 channels with its own
                # scratch plane so two engines fill `chan` concurrently.
                # eng=gpsimd computes rsqrt with the bit-trick seed plus two
                # Newton steps (4.8e-6 rel) from the constant planes; the
                # DVE path uses its accurate reciprocal + ACT sqrt.
                s = slice(lo, hi)
                pool = eng is nc.gpsimd
                x = rpl_sb[:, 0:T][:, s]
                y = rpl_sb[:, T:2 * T][:, s]
                z = rpl_sb[:, 2 * T:3 * T][:, s]

                def ax(i):
                    return aux[:, i * T:(i + 1) * T][:, s]

                xx, yy, zz, s1, r2, mask, inv2, va, vb, t8 = (
                    ax(i) for i in range(10))
                eng.tensor_tensor(xx, x, x, ALU.mult)
                eng.tensor_tensor(yy, y, y, ALU.mult)
                eng.tensor_tensor(zz, z, z, ALU.mult)
                eng.tensor_tensor(s1, xx, yy, ALU.add)
                eng.tensor_tensor(r2, s1, zz, ALU.add)
                # guard r2 == 0 like the reference's safe_r2 by adding
                # 1e-30: absorbed by rounding for any real point (r2 >>
                # 1e-23) and equivalent at the origin, where every
                # monomial numerator is exactly 0 (Pool's ALU only does
                # mult/add/subtract, so no is_equal masking there)
                if pool:
                    eng.tensor_tensor(mask, r2, cpf[:, 2 * T:3 * T][:, s],
                                      ALU.add)
                else:
                    eng.tensor_scalar(mask, r2, 1e-30, None, ALU.add)
                # Pool has no reciprocal/sqrt/shift/pow: these two ops ride
                # on DVE/ACT even for the Pool chunk (a small, bounded
                # cross-engine coupling), the rest stays on `eng`.
                # DVE reciprocal is exact, ACT sqrt ~7e-6 - well under the
                # fp32r rounding (2.4e-4) that dominates the error budget,
                # so no Newton polish is needed.
                nc.vector.reciprocal(inv2, mask)                   # 1/safe_r2
                nc.scalar.sqrt(va, inv2)                           # inv_r

                # chan is t-major interleaved (col = t*NCH + c) so each
                # point-tile's transpose input is one contiguous 10-col slice
                chan_v = chan[:].rearrange("p (t c) -> p c t", c=NCH)
                ch = [chan_v[:, i, :][:, s] for i in range(NCH)]
                eng.tensor_tensor(ch[0], r2, va, ALU.mult)         # radii
                eng.memset(ch[1], 1.0)                             # ones
                eng.tensor_tensor(ch[2], y, va, ALU.mult)          # l=1
                eng.tensor_tensor(ch[3], z, va, ALU.mult)          # l=2
                eng.tensor_tensor(ch[4], x, va, ALU.mult)          # l=3
                e2 = eng2 or eng
                w = aux[:, 10 * T:11 * T][:, s] if eng2 else vb
                e2.tensor_tensor(w, x, y, ALU.mult)
                e2.tensor_tensor(ch[5], w, inv2, ALU.mult)         # xy/r2
                e2.tensor_tensor(w, y, z, ALU.mult)
                e2.tensor_tensor(ch[6], w, inv2, ALU.mult)         # yz/r2
                if pool:
                    e2.tensor_tensor(w, zz, zz, ALU.add)
                    e2.tensor_tensor(w, w, zz, ALU.add)
                    e2.tensor_tensor(w, w, r2, ALU.subtract)
                else:
                    e2.scalar_tensor_tensor(w, zz, 3.0, r2,
                                            ALU.mult, ALU.subtract)
                e2.tensor_tensor(ch[7], w, inv2, ALU.mult)         # (3zz-r2)/r2
                e2.tensor_tensor(w, x, z, ALU.mult)
                e2.tensor_tensor(ch[8], w, inv2, ALU.mult)         # xz/r2
                e2.tensor_tensor(t8, xx, yy, ALU.subtract)
                e2.tensor_tensor(ch[9], t8, inv2, ALU.mult)        # (xx-yy)/r2

            # the first four groups' tiles (the pipeline-fill window) go
            # first on DVE - the serial chain cost is per-op, not per-
            # column; the rest runs 100% on Pool (emitted after the first
            # two MLP stages below), so no early-group transpose can get
            # scheduled on the in-order PE behind the bulk Pool chunk
            compute_chan(0, 3 * TB, nc.vector)

            # warm the PE while DVE computes the fill-window channel
            # planes: the tensor engine ramps 0.65 -> 2.4 GHz only after
            # ~3us of continuous execution, so a chain of dummy transposes
            # gets the pipeline-fill matmuls out of the slow p-states
            with tc.tile_pool(name="warm", bufs=1, space="PSUM") as wps_pool:
                wp = wps_pool.tile([128, 128], F32)
                for _ in range(12):
                    nc.tensor.transpose(wp[:], id_sb[:], id_sb[:])

            # =========================================================
            # Phase 2: main loop over point-tile groups (the first two are
            # half-sized so the pipeline-fill latency is shorter)
            # =========================================================
            with tc.tile_pool(name="tps", bufs=1, space="PSUM") as tps_pool, \
                 tc.tile_pool(name="hps", bufs=1, space="PSUM") as hps_pool, \
                 tc.tile_pool(name="rps", bufs=1, space="PSUM") as rps_pool, \
                 tc.tile_pool(name="yps", bufs=1, space="PSUM") as yps_pool, \
                 tc.tile_pool(name="kps", bufs=2, space="PSUM") as kps_pool, \
                 tc.tile_pool(name="tsb", bufs=2) as tsb_pool, \
                 tc.tile_pool(name="hsb", bufs=2) as hsb_pool, \
                 tc.tile_pool(name="bqp", bufs=2) as bqp_pool, \
                 tc.tile_pool(name="rbp", bufs=2) as rbp_pool, \
                 tc.tile_pool(name="kout", bufs=6) as kout_pool:

                def mlp_stage(t0, nt):
                    # nt transposes: [128, 10] channel blocks -> [10, nt*128]
                    n = nt * 128
                    t_ps = tps_pool.tile([NCH, TB * 128], F32, tag="t_ps")
                    for dt in range(nt):
                        nc.tensor.transpose(
                            t_ps[:, 128 * dt:128 * (dt + 1)],
                            chan[:, NCH * (t0 + dt):NCH * (t0 + dt) + NCH],
                            id_sb[:])
                    t_sb = tsb_pool.tile([NCH, TB * 128], F32R, tag="t_sb")
                    nc.scalar.copy(t_sb[:, :n], t_ps[:, :n])

                    # radial MLP hidden layer, whole group in one matmul
                    h_ps = hps_pool.tile([H, TB * 128], F32, tag="h_ps")
                    nc.tensor.matmul(h_ps[:, :n], w1r_sb[:], t_sb[:, :n],
                                     start=True, stop=True)
                    h_sb = hsb_pool.tile([H, TB * 128], F32R, tag="h_sb")
                    nc.scalar.activation(h_sb[:, :n], h_ps[:, :n], ACTF.Relu,
                                         bias=b1_sb[:])

                    # R (54 rows, path-repeated) and Y' (54 rows)
                    r_ps = rps_pool.tile([PL, TB * 128], F32, tag="r_ps")
                    nc.tensor.matmul(r_ps[:, :n], w2e_sb[:], h_sb[:, :n],
                                     start=True, stop=True)
                    y_ps = yps_pool.tile([PL, TB * 128], F32, tag="y_ps")
                    nc.tensor.matmul(y_ps[:, :n], ey_sb[:], t_sb[:, :n],
                                     start=True, stop=True)

                    # B = (R + b2) * Y', rounded to fp32r. The +b2 rides along
                    # the ACT psum->sbuf drain (DVE can read only one PSUM
                    # operand, so R comes via SBUF)
                    rb_sb = rbp_pool.tile([PL, TB * 128], F32, tag="rb_sb")
                    nc.scalar.activation(rb_sb[:, :n], r_ps[:, :n],
                                         ACTF.Identity, bias=b2_sb[:])
                    bq = bqp_pool.tile([PL, TB * 128], F32R, tag="bq")
                    nc.vector.tensor_tensor(bq[:, :n], rb_sb[:, :n],
                                            y_ps[:, :n], ALU.mult)
                    return bq

                def k_stage(t0, nt, bq):
                    for dt in range(nt):
                        zt = 128 * (t0 + dt)
                        if zt >= ZC:
                            break
                        rows = min(128, ZC - zt)
                        k_ps = kps_pool.tile([128, IJ], F32, tag="k_ps")
                        for half in range(2):
                            nc.tensor.matmul(
                                k_ps[:, 512 * half:512 * (half + 1)],
                                bq[:, 128 * dt:128 * (dt + 1)],
                                m_sb[:, 512 * half:512 * (half + 1)],
                                start=True, stop=True)
                        k_sb = kout_pool.tile([128, IJ], F32, tag="k_sb")
                        # GPSIMD/Pool cannot read PSUM; split drains ACT/DVE
                        if dt % 2 == 0:
                            nc.scalar.copy(k_sb[:], k_ps[:])
                        else:
                            nc.vector.tensor_copy(k_sb[:], k_ps[:])
                        nc.sync.dma_start(out[zt:zt + rows, :], k_sb[0:rows, :])

                # software pipeline, 2 groups deep: the PE<->ACT handoff
                # chain of group g's MLP (transpose -> copy -> h -> relu ->
                # r -> +b2) spans ~6us, slightly more than one store window,
                # so contractions consume bq two groups behind production
                groups = [(0, 2), (2, 2)] + [(t0, TB) for t0 in range(TB, T, TB)]
                bqs = [mlp_stage(*groups[0]), mlp_stage(*groups[1])]
                compute_chan(3 * TB, T, nc.gpsimd)
                for i in range(2, len(groups)):
                    if i <= 3:
                        # pin the pipeline-fill groups ahead of everything
                        # in the list scheduler's priority heap
                        with tc.high_priority():
                            k_stage(*groups[i - 2], bqs[i - 2])
                    else:
                        k_stage(*groups[i - 2], bqs[i - 2])
                    bqs.append(mlp_stage(*groups[i]))
                k_stage(*groups[-2], bqs[-2])
                k_stage(*groups[-1], bqs[-1])
    nc.compile()
    return nc


def _get_program():
    if "nc" not in _CACHE:
        _CACHE["nc"] = _build_program()
    return _CACHE["nc"]


def _host_prep(r, W1, b1, W2, b2, cg, ylm_mix, rf_mix, norm_coef):
    r = np.asarray(r, dtype=np.float32)
    W1 = np.asarray(W1, dtype=np.float32)
    b1 = np.asarray(b1, dtype=np.float32)
    W2 = np.asarray(W2, dtype=np.float32)
    b2 = np.asarray(b2, dtype=np.float32)
    cg = np.asarray(cg, dtype=np.float32)
    ylm_mix = np.asarray(ylm_mix, dtype=np.float32)
    rf_mix = np.asarray(rf_mix, dtype=np.float32)
    norm_coef = np.asarray(norm_coef, dtype=np.float32)

    # constant folding: M[p*9+l, ij] = sum_k rf[k,p] ylm_s[k,l] cg[k,ij] nc0[ij]
    ylm_s = ylm_mix.astype(np.float64) * YLM_SCALE[None, :]
    w_kpl = (rf_mix.astype(np.float64)[:, :, None] * ylm_s[:, None, :])
    m = w_kpl.reshape(KDIM, PL).T @ cg.astype(np.float64).reshape(KDIM, IJ)
    m *= norm_coef[:, :, 0].astype(np.float64).reshape(1, IJ)
    md = np.ascontiguousarray(m.astype(np.float32))

    w1r = np.zeros((NCH, 128), dtype=np.float32)
    w1r[0, :] = W1[0]
    eyd = np.zeros((NCH, PL), dtype=np.float32)
    for l in range(9):
        for p in range(NPATH):
            eyd[1 + l, p * 9 + l] = 1.0

    shared = {
        "w1r": w1r,
        "eyd": eyd,
        "w2e": np.ascontiguousarray(np.repeat(W2, 9, axis=1)),
        "b1c": np.ascontiguousarray(b1.reshape(H, 1)),
        "b2r": np.ascontiguousarray(np.repeat(b2, 9).reshape(PL, 1)),
        "md": md,
        "identd": np.eye(128, dtype=np.float32),
    }

    in_maps = []
    for c in range(NCORES):
        rs = r[c * ZC:(c + 1) * ZC]
        rp = np.empty((ZC_PAD, 3), dtype=np.float32)
        rp[:ZC] = rs
        rp[ZC:] = np.array([1.0, 0.0, 0.0], dtype=np.float32)
        rpl = rp.reshape(T, 128, 3).transpose(1, 2, 0).reshape(128, 3 * T)
        m = dict(shared)
        m["rpl"] = np.ascontiguousarray(rpl)
        in_maps.append(m)
    return in_maps


def _run_device(in_maps, trace=False, **kw):
    nc = _get_program()
    return run_bass_kernel_spmd(nc, in_maps, core_ids=list(range(NCORES)),
                                trace=trace, **kw)


def kernel(r, W1, b1, W2, b2, cg, ylm_mix, rf_mix, norm_coef):
    r = np.asarray(r, dtype=np.float32)
    norm_coef_f = np.asarray(norm_coef, dtype=np.float32)
    in_maps = _host_prep(r, W1, b1, W2, b2, cg, ylm_mix, rf_mix, norm_coef_f)
    res = _run_device(in_maps)
    out = np.concatenate([res.results[c]["out"] for c in range(NCORES)], axis=0)

    # points with exactly zero radius use norm_coef[..., 1] instead of [..., 0]
    x, y, z = r[:, 0], r[:, 1], r[:, 2]
    r2 = (x * x + y * y) + z * z
    zero = r2 == np.float32(0.0)
    if np.any(zero):
        scale = (norm_coef_f[:, :, 1].astype(np.float64)
                 / norm_coef_f[:, :, 0].astype(np.float64)).reshape(1, IJ)
        out[zero] = (out[zero].astype(np.float64) * scale).astype(np.float32)

    return out.reshape(Z, DO, DI)
